# revision 3
# baseline (speedup 1.0000x reference)
"""Trainium2 Bass kernel for nn_Cross_Message (GNN message passing).

Strategy (8 NeuronCores, SPMD):
  - Host: relabel source nodes by degree (descending) into 392 groups of 128;
    deal groups round-robin to the 8 cores (49 groups each) so every core runs
    the same compile-time column schedule Ksched[i]. Each node owns one SBUF
    partition of its group; its edges occupy that partition's column slots.
    Per-source softmax + weighted segment-sum become per-partition ops with
    no cross-core communication (each core owns disjoint output rows).
  - Host performs the edge gather: X_h_2 rows are laid out in slot order,
    partition-major, in bf16 (x2s), so the device streams each group with one
    contiguous HWDGE DMA at line rate (no indirect gather, no SWDGE
    descriptor-ring serialization). Column 0 of every group block carries the
    group's X_h_1 rows, so dot(col0, col0) doubles as |x1|^2.
  - Device, per quad of 4 groups: per-column dots via DVE tensor_tensor_reduce
    against column 0; |x2|^2 via ACT Square (a filler function present in
    every activation table set -> the whole kernel needs ONE table load),
    either per-column with accumulate or one big Square + DVE segmented
    reduce (picked per quad for DVE/ACT balance); cosine normalization via
    fast inverse-sqrt (bit trick + Newton) on DVE; masked softmax via one
    batched ACT Exp; weighted aggregation on the TENSOR engine as accumulating
    matmuls diag(w_k) @ X2_k, with the diagonals built by GPSIMD
    tensor_scalar ops (idle engine); gate sigmoid via Tanh (same table set)
    with matmul+tanh+affine batched per quad.
  - Host: inverse-permute the 8 per-core outputs into the full [N1, 128].

Self-contained: imports numpy + ml_dtypes + concourse only.
"""
import os
import sys

import numpy as np

for _p in ("/opt/trn_rl_repo", "/root/.axon_site/_ro/trn_rl_repo"):
    if os.path.isdir(_p) and _p not in sys.path:
        sys.path.append(_p)

N1 = 50000
N2 = 50000
E = 640000
D = 128      # node feature dim
A = 64       # attr dim
P = 128      # partitions
NCORES = 8
G = 392      # groups (392*128 = 50176 >= N1)
GPC = G // NCORES
QUAD = 4
EPS = 1e-8
MASKNEG = -60.0
TINY = 1e-30
MAGIC = 0x5F3759DF
POOL_DOT_FRAC = 0.2  # fraction of dot columns on gpsimd
SQCOLS_ACT = 7  # leading cols per group squared per-k on ACT  # quads with avg K >= this use big-square + DVE reduce

LAST_EXEC_NS = None
LAST_RES = None


def _bf16(x):
    import ml_dtypes
    return np.asarray(x, dtype=ml_dtypes.bfloat16)


def _prep(X_h_1, X_h_2, X_n_1, cross_indices, W_gate):
    src = np.asarray(cross_indices[0], dtype=np.int64)
    dst = np.asarray(cross_indices[1], dtype=np.int64)
    X_h_1 = np.asarray(X_h_1, dtype=np.float32)
    X_h_2 = np.asarray(X_h_2, dtype=np.float32)
    X_n_1 = np.asarray(X_n_1, dtype=np.float32)
    W_gate = np.asarray(W_gate, dtype=np.float32)

    deg = np.bincount(src, minlength=N1).astype(np.int64)
    node_order = np.argsort(-deg, kind="stable")
    node_order_p = np.full(G * P, -1, dtype=np.int64)
    node_order_p[:N1] = node_order
    deg_p = np.where(node_order_p >= 0, deg[np.clip(node_order_p, 0, N1 - 1)], 0)

    Kg = deg_p.reshape(G, P).max(axis=1)
    Ksched = Kg.reshape(GPC, NCORES).max(axis=1).astype(np.int64)
    # widths including the x1 column
    W = Ksched + 1
    sumW = int(W.sum())

    eorder = np.argsort(src, kind="stable")
    dst_sorted = dst[eorder]
    off = np.zeros(N1 + 1, dtype=np.int64)
    off[1:] = np.cumsum(deg)

    X_h_1b = _bf16(X_h_1)
    X_h_2b = _bf16(X_h_2)

    per_core = []
    for c in range(NCORES):
        mneg_all = np.full((P, sumW), MASKNEG, dtype=np.float32)
        xnt = np.zeros((P, GPC * P), dtype=np.float32)
        # host-side gather, partition-major: group block = [x1 | K slot rows]
        x2s = np.zeros((P, sumW * D), dtype=X_h_2b.dtype)
        woff = 0
        for i in range(GPC):
            g = i * NCORES + c
            K = int(Ksched[i])
            nodes = node_order_p[g * P:(g + 1) * P]
            degs = deg_p[g * P:(g + 1) * P]
            vn = nodes >= 0
            x1blk = np.zeros((P, D), dtype=X_h_1b.dtype)
            x1blk[vn] = X_h_1b[nodes[vn]]
            x2s[:, woff * D:(woff + 1) * D] = x1blk
            if K > 0:
                col = np.arange(K)[None, :]
                valid = col < degs[:, None]
                base = np.where(vn, off[np.clip(nodes, 0, N1 - 1)], 0)
                epos = base[:, None] + col
                blk_idx = np.zeros((P, K), dtype=np.int64)
                blk_idx[valid] = dst_sorted[np.clip(epos, 0, E - 1)][valid]
                x2s[:, (woff + 1) * D:(woff + 1 + K) * D] = (
                    X_h_2b[blk_idx].reshape(P, K * D))
                mneg_all[:, woff + 1:woff + 1 + K][valid] = 0.0
            woff += K + 1
            xnt[:A, i * P:(i + 1) * P][:, vn] = X_n_1[nodes[vn]].T
        per_core.append(dict(mneg_all=mneg_all, xnt=_bf16(xnt), x2s=x2s))

    wgt = np.zeros((P, P), dtype=np.float32)
    wgt[:A, :] = W_gate.T

    meta = dict(Ksched=tuple(int(k) for k in Ksched), node_order_p=node_order_p,
                deg=deg, wgt=_bf16(wgt), sumW=sumW, idm=_bf16(np.eye(P)))
    return per_core, meta


def _build(Ksched, sumW):
    import concourse.bass as bass
    import concourse.mybir as mybir
    from concourse import bacc
    from concourse.tile import TileContext

    f32 = mybir.dt.float32
    bf16 = mybir.dt.bfloat16
    i32 = mybir.dt.int32
    AF = mybir.ActivationFunctionType
    ALU = mybir.AluOpType

    Wd = [k + 1 for k in Ksched]
    # chunks of consecutive groups, capped by total width so early
    # high-degree groups form small chunks (smoother pipeline, smaller
    # x2 tiles) and late low-degree groups batch up to QUAD per chunk
    CAP = 50
    quads = []
    cur = []
    curw = 0
    for g in range(GPC):
        if cur and (curw + Wd[g] > CAP or len(cur) >= QUAD):
            quads.append(cur)
            cur = []
            curw = 0
        cur.append(g)
        curw += Wd[g]
    if cur:
        quads.append(cur)
    woffs = [0]
    for w in Wd:
        woffs.append(woffs[-1] + w)
    Kmax = max(Ksched)

    nc = bacc.Bacc()
    x2s = nc.dram_tensor("x2s", [P, sumW * D], bf16, kind="ExternalInput")
    mnegs = nc.dram_tensor("mnegs", [P, sumW], f32, kind="ExternalInput")
    xnt = nc.dram_tensor("xnt", [P, GPC * P], bf16, kind="ExternalInput")
    wgt = nc.dram_tensor("wgt", [P, P], bf16, kind="ExternalInput")
    idmd = nc.dram_tensor("idmd", [P, P], bf16, kind="ExternalInput")
    out = nc.dram_tensor("out", [GPC * P, D], f32, kind="ExternalOutput")

    with TileContext(nc) as tc:
        with (
            tc.tile_pool(name="const", bufs=1) as cp,
            tc.tile_pool(name="sb", bufs=2) as sb,
            tc.tile_pool(name="x2p", bufs=6) as x2p,
            tc.tile_pool(name="sq", bufs=4) as sqp,
            tc.tile_pool(name="dg", bufs=3) as dg,
            tc.tile_pool(name="wqp", bufs=5) as wqp,
            tc.tile_pool(name="nsqp", bufs=3) as nsqp,
            tc.tile_pool(name="ps", bufs=5, space="PSUM") as ps,
            tc.tile_pool(name="psg", bufs=2, space="PSUM") as psg,
        ):
            wgt_sb = cp.tile([P, P], bf16)
            nc.sync.dma_start(out=wgt_sb[:], in_=wgt[:, :])
            idm = cp.tile([P, P], bf16)
            nc.sync.dma_start(out=idm[:], in_=idmd[:, :])
            neg1 = cp.tile([P, 1], f32)
            nc.vector.memset(neg1[:], -1.0)
            gates = cp.tile([P, GPC * P], bf16)
            mneg_all = cp.tile([P, sumW], f32)
            nc.sync.dma_start(out=mneg_all[:], in_=mnegs[:, :])
            xnt_all = cp.tile([P, GPC * P], bf16)
            nc.sync.dma_start(out=xnt_all[:], in_=xnt[:, :])

            stateA = {}
            stateB = {}
            stateC = {}

            def stage_sq(qi):
                groups, x2q, _ = stateA[qi]
                q0 = groups[0]
                base = woffs[q0]
                Wq = woffs[groups[-1] + 1] - base
                nsqq = nsqp.tile([P, Wq], f32, tag="nsqq")
                scr2 = sb.tile([P, D], bf16, tag="scr2")
                sqtiles = {}
                for g in groups:
                    go = woffs[g] - base
                    w = Wd[g]
                    m = min(w, SQCOLS_ACT)
                    for c in range(m):
                        nc.scalar.activation(
                            out=scr2[:],
                            in_=x2q[:, (go + c) * D:(go + c + 1) * D],
                            func=AF.Square,
                            accum_out=nsqq[:, go + c:go + c + 1])
                    if m < w:
                        sqs = sqp.tile([P, (Kmax + 1) * D], bf16, tag="sqs")
                        nc.scalar.activation(
                            out=sqs[:, 0:(w - m) * D],
                            in_=x2q[:, (go + m) * D:(go + w) * D],
                            func=AF.Square)
                        sqtiles[g] = sqs
                stateA[qi] = (groups, x2q, (nsqq, sqtiles))

            def stage_compute(qi):
                groups, x2q, (nsqq, sqtiles) = stateA.pop(qi)
                q0 = groups[0]
                base = woffs[q0]
                Wq = woffs[groups[-1] + 1] - base

                dotq = sb.tile([P, Wq], f32, tag="dotq")
                scrb = sb.tile([P, D], bf16, tag="scrb")
                for g in groups:
                    go = woffs[g] - base
                    w = Wd[g]
                    x1c = x2q[:, go * D:(go + 1) * D]
                    for c in range(w):
                        nc.vector.scalar_tensor_tensor(
                            out=scrb[:],
                            in0=x2q[:, (go + c) * D:(go + c + 1) * D],
                            scalar=0.0, in1=x1c,
                            op0=ALU.bypass, op1=ALU.mult,
                            accum_out=dotq[:, go + c:go + c + 1])
                    if g in sqtiles:
                        m = min(w, SQCOLS_ACT)
                        nc.vector.tensor_reduce(
                            out=nsqq[:, go + m:go + w],
                            in_=sqtiles[g][:, 0:(w - m) * D].rearrange(
                                "p (k d) -> p k d", d=D),
                            axis=mybir.AxisListType.X, op=ALU.add)

                # rn12 = 1/sqrt(clamp(nsq1)*clamp(nsq2)), fast inverse sqrt
                nc.vector.tensor_scalar_max(out=nsqq[:], in0=nsqq[:],
                                            scalar1=float(EPS * EPS))
                n12 = sb.tile([P, Wq], f32, tag="n12")
                for g in groups:
                    go = woffs[g] - base
                    w = Wd[g]
                    nc.vector.tensor_scalar_mul(
                        out=n12[:, go:go + w], in0=nsqq[:, go:go + w],
                        scalar1=nsqq[:, go:go + 1])
                y = sb.tile([P, Wq], f32, tag="y")
                ti = sb.tile([P, Wq], i32, tag="ti")
                nc.vector.tensor_scalar(
                    out=ti[:], in0=n12[:].bitcast(i32), scalar1=1,
                    scalar2=None, op0=ALU.arith_shift_right)
                nc.vector.tensor_scalar(
                    out=y[:].bitcast(i32), in0=ti[:], scalar1=MAGIC,
                    scalar2=-1, op0=ALU.subtract, op1=ALU.mult)
                t2 = sb.tile([P, Wq], f32, tag="t2")
                for _ in range(1):
                    nc.vector.tensor_tensor(out=t2[:], in0=y[:], in1=y[:],
                                            op=ALU.mult)
                    nc.vector.tensor_tensor(out=t2[:], in0=t2[:], in1=n12[:],
                                            op=ALU.mult)
                    nc.vector.tensor_scalar(out=t2[:], in0=t2[:], scalar1=-0.5,
                                            scalar2=1.5, op0=ALU.mult,
                                            op1=ALU.add)
                    nc.vector.tensor_tensor(out=y[:], in0=y[:], in1=t2[:],
                                            op=ALU.mult)

                sim = sb.tile([P, Wq], f32, tag="sim")
                nc.vector.tensor_tensor(out=sim[:], in0=dotq[:], in1=y[:],
                                        op=ALU.mult)
                nc.vector.tensor_tensor(out=sim[:], in0=sim[:],
                                        in1=mneg_all[:, base:base + Wq],
                                        op=ALU.add)
                ex = sb.tile([P, Wq], f32, tag="ex")
                nc.scalar.activation(out=ex[:], in_=sim[:], func=AF.Exp,
                                     bias=neg1[:], scale=1.0)
                S = sb.tile([P, len(groups)], f32, tag="S")
                for n, g in enumerate(groups):
                    go = woffs[g] - base
                    w = Wd[g]
                    nc.vector.tensor_reduce(
                        out=S[:, n:n + 1],
                        in_=ex[:, go:go + w].rearrange("p (o w) -> p o w", o=1),
                        axis=mybir.AxisListType.X, op=ALU.add)
                nc.vector.tensor_scalar_add(out=S[:], in0=S[:],
                                            scalar1=float(TINY))
                r = sb.tile([P, len(groups)], f32, tag="r")
                nc.vector.reciprocal(out=r[:], in_=S[:])
                wq = wqp.tile([P, Wq], f32, tag="wq")
                for n, g in enumerate(groups):
                    go = woffs[g] - base
                    w = Wd[g]
                    nc.vector.tensor_scalar_mul(
                        out=wq[:, go:go + w], in0=ex[:, go:go + w],
                        scalar1=r[:, n:n + 1])
                stateB[qi] = (groups, x2q, wq)

            def stage_mac(qi):
                groups, x2q, wq = stateB.pop(qi)
                q0 = groups[0]
                base = woffs[q0]
                ng = len(groups)
                agg = ps.tile([P, ng * D], f32, space="PSUM")
                # start=True clears the WHOLE PSUM bank, so only the very
                # first matmul into this tile may use it; later groups rely
                # on per-element has_written (first write overwrites).
                nmm = sum(max(Ksched[g], 1) for g in groups)
                mi = 0
                for n, g in enumerate(groups):
                    go = woffs[g] - base
                    K = Ksched[g]
                    if K == 0:
                        dk0 = dg.tile([P, P], bf16, tag="dk0")
                        nc.vector.memset(dk0[:], 0.0)
                        nc.tensor.matmul(agg[:, n * D:(n + 1) * D], lhsT=dk0[:],
                                         rhs=x2q[:, go * D:(go + 1) * D],
                                         start=(mi == 0), stop=(mi == nmm - 1))
                        mi += 1
                        continue
                    dk = dg.tile([P, K * P], bf16, tag="dk")
                    nc.gpsimd.tensor_tensor(
                        out=dk[:].rearrange("p (k q) -> p k q", q=P),
                        in0=idm[:].rearrange("p (o q) -> p o q", o=1
                                             ).broadcast_to((P, K, P)),
                        in1=wq[:, go + 1:go + 1 + K].rearrange(
                            "p (k o) -> p k o", o=1).broadcast_to((P, K, P)),
                        op=ALU.mult)
                    for k in range(1, K + 1):
                        nc.tensor.matmul(agg[:, n * D:(n + 1) * D],
                                         lhsT=dk[:, (k - 1) * P:k * P],
                                         rhs=x2q[:, (go + k) * D:(go + k + 1) * D],
                                         start=(mi == 0), stop=(mi == nmm - 1))
                        mi += 1
                stateC[qi] = (groups, agg)

            def stage_out(qi):
                groups, agg = stateC.pop(qi)
                q0 = groups[0]
                ng = len(groups)
                out_sb = sb.tile([P, ng * D], f32, tag="outt")
                nc.vector.tensor_tensor(
                    out=out_sb[:], in0=agg[:],
                    in1=gates[:, q0 * P:(q0 + ng) * P], op=ALU.mult)
                nc.sync.dma_start(
                    out=out[q0 * P:(q0 + ng) * P, :].rearrange(
                        "(n p) d -> p n d", p=P),
                    in_=out_sb[:].rearrange("p (n d) -> p n d", d=D))

            for qi, groups in enumerate(quads):
                q0 = groups[0]
                ng = len(groups)
                # gate unit: sigmoid(x) = 0.5*tanh(x/2) + 0.5, batched
                gps = psg.tile([P, ng * P], f32, space="PSUM")
                for n, g in enumerate(groups):
                    nc.tensor.matmul(gps[:, n * P:(n + 1) * P],
                                     lhsT=xnt_all[:, g * P:(g + 1) * P],
                                     rhs=wgt_sb[:], start=True, stop=True)
                ge = sb.tile([P, ng * P], bf16, tag="ge")
                nc.scalar.activation(out=ge[:], in_=gps[:], func=AF.Tanh,
                                     bias=0.0, scale=0.5)
                nc.vector.tensor_scalar(out=gates[:, q0 * P:(q0 + ng) * P],
                                        in0=ge[:], scalar1=0.5, scalar2=0.5,
                                        op0=ALU.mult, op1=ALU.add)

                base = woffs[q0]
                Wq = woffs[groups[-1] + 1] - base
                x2q = x2p.tile([P, Wq * D], bf16, tag="x2")
                nc.sync.dma_start(out=x2q[:],
                                  in_=x2s[:, base * D:(base + Wq) * D])
                stateA[qi] = (groups, x2q, None)
                if qi - 1 in stateA:
                    stage_compute(qi - 1)
                if qi - 2 in stateB:
                    stage_mac(qi - 2)
                stage_sq(qi)
                if qi - 4 in stateC:
                    stage_out(qi - 4)
            nq = len(quads)
            if nq - 1 in stateA:
                stage_compute(nq - 1)
            for j in (nq - 4, nq - 3, nq - 2, nq - 1):
                if j in stateB:
                    stage_mac(j)
                if j in stateC:
                    stage_out(j)
    nc.compile()
    return nc


def kernel(X_h_1, X_h_2, X_n_1, cross_indices, W_gate):
    global LAST_EXEC_NS
    from concourse.bass_utils import run_bass_kernel_spmd

    per_core, meta = _prep(X_h_1, X_h_2, X_n_1, cross_indices, W_gate)
    nc = _build(meta["Ksched"], meta["sumW"])

    in_maps = []
    for c in range(NCORES):
        pc = per_core[c]
        in_maps.append(dict(x2s=pc["x2s"], mnegs=pc["mneg_all"],
                            xnt=pc["xnt"], wgt=meta["wgt"], idmd=meta["idm"]))

    trace = bool(int(os.environ.get("BASS_KERNEL_TRACE", "0")))
    try:
        res = run_bass_kernel_spmd(nc, in_maps, list(range(NCORES)),
                                   trace=trace)
    except ModuleNotFoundError:
        res = run_bass_kernel_spmd(nc, in_maps, list(range(NCORES)),
                                   trace=False)
    LAST_EXEC_NS = res.exec_time_ns
    globals()["LAST_RES"] = res

    node_order_p = meta["node_order_p"]
    deg = meta["deg"]
    out_full = np.zeros((N1, D), dtype=np.float32)
    for c in range(NCORES):
        rows = res.results[c]["out"]
        for i in range(GPC):
            g = i * NCORES + c
            nodes = node_order_p[g * P:(g + 1) * P]
            vn = nodes >= 0
            out_full[nodes[vn]] = rows[i * P:(i + 1) * P][vn]
    out_full[deg == 0] = 0.0
    return out_full



# revision 4
# speedup vs baseline: 1.0517x; 1.0517x over previous
"""Trainium2 Bass kernel for nn_Cross_Message (GNN message passing).

Strategy (8 NeuronCores, SPMD):
  - Host: relabel source nodes by degree (descending) into 392 groups of 128;
    deal groups round-robin to the 8 cores (49 groups each) so every core runs
    the same compile-time column schedule Ksched[i]. Each node owns one SBUF
    partition of its group; its edges occupy that partition's column slots.
    Per-source softmax + weighted segment-sum become per-partition ops with
    no cross-core communication (each core owns disjoint output rows).
  - Host performs the edge gather: X_h_2 rows are laid out in slot order,
    partition-major, in bf16 (x2s), so the device streams each group with one
    contiguous HWDGE DMA at line rate. Column 0 of every group block carries
    the group's X_h_1 rows.
  - Device, per chunk of consecutive groups (<= ~48 columns): one batched
    bf16 product pass (x2 * x1-broadcast, DVE 2x mode), one batched ACT
    Square pass, then shared deep fold-trees (pairwise halving adds at DVE
    2x mode, f32 segmented-reduce tail) produce per-edge dots and |x|^2 in
    wide [P, C] tiles.  The cosine/softmax chain runs chunk-wide (clamp,
    per-group n1*n2, fast inverse sqrt, mask, one batched Exp, per-group
    sums + fast reciprocal + weight scale).  Weighted aggregation stays on
    the TENSOR engine as accumulating diag(w_k) @ X2_k matmuls with diagonals
    built by GPSIMD; gate sigmoid via Tanh batched per chunk.
  - Host: inverse-permute the 8 per-core outputs into the full [N1, 128].

Self-contained: imports numpy + ml_dtypes + concourse only.
"""
import os
import sys

import numpy as np

for _p in ("/opt/trn_rl_repo", "/root/.axon_site/_ro/trn_rl_repo"):
    if os.path.isdir(_p) and _p not in sys.path:
        sys.path.append(_p)

N1 = 50000
N2 = 50000
E = 640000
D = 128      # node feature dim
A = 64       # attr dim
P = 128      # partitions
NCORES = 8
G = 392      # groups (392*128 = 50176 >= N1)
GPC = G // NCORES
EPS = 1e-8
MASKNEG = -60.0
TINY = 1e-30
MAGIC = 0x5F3759DF
CAP = 48     # max columns per chunk
MAXG = 4     # max groups per chunk

LAST_EXEC_NS = None
LAST_RES = None


def _bf16(x):
    import ml_dtypes
    return np.asarray(x, dtype=ml_dtypes.bfloat16)


def _prep(X_h_1, X_h_2, X_n_1, cross_indices, W_gate):
    src = np.asarray(cross_indices[0], dtype=np.int64)
    dst = np.asarray(cross_indices[1], dtype=np.int64)
    X_h_1 = np.asarray(X_h_1, dtype=np.float32)
    X_h_2 = np.asarray(X_h_2, dtype=np.float32)
    X_n_1 = np.asarray(X_n_1, dtype=np.float32)
    W_gate = np.asarray(W_gate, dtype=np.float32)

    deg = np.bincount(src, minlength=N1).astype(np.int64)
    node_order = np.argsort(-deg, kind="stable")
    node_order_p = np.full(G * P, -1, dtype=np.int64)
    node_order_p[:N1] = node_order
    deg_p = np.where(node_order_p >= 0, deg[np.clip(node_order_p, 0, N1 - 1)], 0)

    Kg = deg_p.reshape(G, P).max(axis=1)
    Ksched = Kg.reshape(GPC, NCORES).max(axis=1).astype(np.int64)
    # widths including the x1 column
    W = Ksched + 1
    sumW = int(W.sum())

    eorder = np.argsort(src, kind="stable")
    dst_sorted = dst[eorder]
    off = np.zeros(N1 + 1, dtype=np.int64)
    off[1:] = np.cumsum(deg)

    X_h_1b = _bf16(X_h_1)
    X_h_2b = _bf16(X_h_2)

    per_core = []
    for c in range(NCORES):
        mneg_all = np.full((P, sumW), MASKNEG, dtype=np.float32)
        xnt = np.zeros((P, GPC * P), dtype=np.float32)
        # host-side gather, partition-major: group block = [x1 | K slot rows]
        x2s = np.zeros((P, sumW * D), dtype=X_h_2b.dtype)
        woff = 0
        for i in range(GPC):
            g = i * NCORES + c
            K = int(Ksched[i])
            nodes = node_order_p[g * P:(g + 1) * P]
            degs = deg_p[g * P:(g + 1) * P]
            vn = nodes >= 0
            x1blk = np.zeros((P, D), dtype=X_h_1b.dtype)
            x1blk[vn] = X_h_1b[nodes[vn]]
            x2s[:, woff * D:(woff + 1) * D] = x1blk
            if K > 0:
                col = np.arange(K)[None, :]
                valid = col < degs[:, None]
                base = np.where(vn, off[np.clip(nodes, 0, N1 - 1)], 0)
                epos = base[:, None] + col
                blk_idx = np.zeros((P, K), dtype=np.int64)
                blk_idx[valid] = dst_sorted[np.clip(epos, 0, E - 1)][valid]
                x2s[:, (woff + 1) * D:(woff + 1 + K) * D] = (
                    X_h_2b[blk_idx].reshape(P, K * D))
                mneg_all[:, woff + 1:woff + 1 + K][valid] = 0.0
            woff += K + 1
            xnt[:A, i * P:(i + 1) * P][:, vn] = X_n_1[nodes[vn]].T
        per_core.append(dict(mneg_all=mneg_all, xnt=_bf16(xnt), x2s=x2s))

    wgt = np.zeros((P, P), dtype=np.float32)
    wgt[:A, :] = W_gate.T
    meta = dict(Ksched=tuple(int(k) for k in Ksched), node_order_p=node_order_p,
                deg=deg, wgt=_bf16(wgt), sumW=sumW, idm=_bf16(np.eye(P)))
    return per_core, meta


def _build(Ksched, sumW):
    import concourse.bass as bass
    import concourse.mybir as mybir
    from concourse import bacc
    from concourse.tile import TileContext

    f32 = mybir.dt.float32
    bf16 = mybir.dt.bfloat16
    i32 = mybir.dt.int32
    AF = mybir.ActivationFunctionType
    ALU = mybir.AluOpType

    Wd = [k + 1 for k in Ksched]
    # chunks of consecutive groups, capped by width and group count
    chunks = []
    cur = []
    curw = 0
    for g in range(GPC):
        if cur and (curw + Wd[g] > CAP or len(cur) >= MAXG):
            chunks.append(cur)
            cur = []
            curw = 0
        cur.append(g)
        curw += Wd[g]
    if cur:
        chunks.append(cur)
    woffs = [0]
    for w in Wd:
        woffs.append(woffs[-1] + w)
    Kmax = max(Ksched)
    Cmax = max(woffs[ch[-1] + 1] - woffs[ch[0]] for ch in chunks)

    nc = bacc.Bacc()
    x2s = nc.dram_tensor("x2s", [P, sumW * D], bf16, kind="ExternalInput")
    mnegs = nc.dram_tensor("mnegs", [P, sumW], f32, kind="ExternalInput")
    xnt = nc.dram_tensor("xnt", [P, GPC * P], bf16, kind="ExternalInput")
    wgt = nc.dram_tensor("wgt", [P, P], bf16, kind="ExternalInput")
    idmd = nc.dram_tensor("idmd", [P, P], bf16, kind="ExternalInput")
    out = nc.dram_tensor("out", [GPC * P, D], f32, kind="ExternalOutput")

    with TileContext(nc) as tc:
        with (
            tc.tile_pool(name="const", bufs=1) as cp,
            tc.tile_pool(name="sb", bufs=2) as sb,
            tc.tile_pool(name="x2p", bufs=3) as x2p,
            tc.tile_pool(name="sqp", bufs=2) as sqp,
            tc.tile_pool(name="prp", bufs=2) as prp,
            tc.tile_pool(name="fld", bufs=2) as fld,
            tc.tile_pool(name="dg", bufs=3) as dg,
            tc.tile_pool(name="wqp", bufs=4) as wqp,
            tc.tile_pool(name="ps", bufs=4, space="PSUM") as ps,
            tc.tile_pool(name="psg", bufs=2, space="PSUM") as psg,
        ):
            wgt_sb = cp.tile([P, P], bf16)
            nc.sync.dma_start(out=wgt_sb[:], in_=wgt[:, :])
            idm = cp.tile([P, P], bf16)
            nc.sync.dma_start(out=idm[:], in_=idmd[:, :])
            neg1 = cp.tile([P, 1], f32)
            nc.vector.memset(neg1[:], -1.0)
            gates = cp.tile([P, GPC * P], bf16)
            mneg_all = cp.tile([P, sumW], f32)
            nc.sync.dma_start(out=mneg_all[:], in_=mnegs[:, :])
            xnt_all = cp.tile([P, GPC * P], bf16)
            nc.sync.dma_start(out=xnt_all[:], in_=xnt[:, :])

            stA = {}
            stB = {}
            stC = {}

            def stage_front(ci):
                """DMA done -> products, squares, folds, chunk softmax, wq."""
                groups, x2q = stA.pop(ci)
                g0 = groups[0]
                base = woffs[g0]
                C = woffs[groups[-1] + 1] - base

                # batched ACT square of the whole chunk (bf16 out)
                sqc = sqp.tile([P, Cmax * D], bf16, tag="sqc")
                nc.scalar.activation(out=sqc[:, 0:C * D], in_=x2q[:, 0:C * D],
                                     func=AF.Square)
                # per-group product pass vs x1 (col 0 of the group block)
                prc = prp.tile([P, Cmax * D], bf16, tag="prc")
                for g in groups:
                    go = woffs[g] - base
                    w = Wd[g]
                    x1c = x2q[:, go * D:(go + 1) * D]
                    nc.vector.tensor_tensor(
                        out=prc[:, go * D:(go + w) * D].rearrange(
                            "p (k d) -> p k d", d=D),
                        in0=x2q[:, go * D:(go + w) * D].rearrange(
                            "p (k d) -> p k d", d=D),
                        in1=x1c.rearrange("p (o d) -> p o d", o=1
                                          ).broadcast_to((P, w, D)),
                        op=ALU.mult)

                # shared fold-trees: 128 -> 8 in bf16 (2x), f32 segred tail
                def fold_tree(src_t, tag):
                    t64 = fld.tile([P, Cmax * 64], bf16, tag=tag + "64")
                    v = src_t[:, 0:C * D].rearrange("p (k d) -> p k d", d=D)
                    nc.vector.tensor_tensor(
                        out=t64[:, 0:C * 64].rearrange("p (k d) -> p k d", d=64),
                        in0=v[:, :, 0:64], in1=v[:, :, 64:128], op=ALU.add)
                    t16 = fld.tile([P, Cmax * 16], bf16, tag=tag + "16")
                    v64 = t64[:, 0:C * 64].rearrange("p (k d) -> p k d", d=64)
                    t32 = fld.tile([P, Cmax * 32], bf16, tag=tag + "32")
                    nc.vector.tensor_tensor(
                        out=t32[:, 0:C * 32].rearrange("p (k d) -> p k d", d=32),
                        in0=v64[:, :, 0:32], in1=v64[:, :, 32:64], op=ALU.add)
                    v32 = t32[:, 0:C * 32].rearrange("p (k d) -> p k d", d=32)
                    nc.vector.tensor_tensor(
                        out=t16[:, 0:C * 16].rearrange("p (k d) -> p k d", d=16),
                        in0=v32[:, :, 0:16], in1=v32[:, :, 16:32], op=ALU.add)
                    red = sb.tile([P, Cmax], f32, tag=tag + "red")
                    nc.vector.tensor_reduce(
                        out=red[:, 0:C],
                        in_=t16[:, 0:C * 16].rearrange("p (k d) -> p k d", d=16),
                        axis=mybir.AxisListType.X, op=ALU.add)
                    return red

                sqw = fold_tree(sqc, "sq")
                dots = fold_tree(prc, "pr")

                # clamp |x|^2 >= eps^2 (covers n1 and n2)
                nc.vector.tensor_scalar_max(out=sqw[:, 0:C], in0=sqw[:, 0:C],
                                            scalar1=float(EPS * EPS))
                # n12 = nsq2 * nsq1(col0 of each group)
                n12 = sb.tile([P, Cmax], f32, tag="n12")
                for g in groups:
                    go = woffs[g] - base
                    w = Wd[g]
                    nc.vector.tensor_scalar_mul(
                        out=n12[:, go:go + w], in0=sqw[:, go:go + w],
                        scalar1=sqw[:, go:go + 1])
                # y = 1/sqrt(n12): fast inverse sqrt + 1 Newton step
                y = sb.tile([P, Cmax], f32, tag="y")
                ti = sb.tile([P, Cmax], i32, tag="ti")
                nc.vector.tensor_scalar(
                    out=ti[:, 0:C], in0=n12[:, 0:C].bitcast(i32), scalar1=1,
                    scalar2=None, op0=ALU.arith_shift_right)
                nc.vector.tensor_scalar(
                    out=y[:, 0:C].bitcast(i32), in0=ti[:, 0:C], scalar1=MAGIC,
                    scalar2=-1, op0=ALU.subtract, op1=ALU.mult)
                t2 = sb.tile([P, Cmax], f32, tag="t2")
                nc.vector.tensor_tensor(out=t2[:, 0:C], in0=y[:, 0:C],
                                        in1=y[:, 0:C], op=ALU.mult)
                nc.vector.tensor_tensor(out=t2[:, 0:C], in0=t2[:, 0:C],
                                        in1=n12[:, 0:C], op=ALU.mult)
                nc.vector.tensor_scalar(out=t2[:, 0:C], in0=t2[:, 0:C],
                                        scalar1=-0.5, scalar2=1.5,
                                        op0=ALU.mult, op1=ALU.add)
                nc.vector.tensor_tensor(out=y[:, 0:C], in0=y[:, 0:C],
                                        in1=t2[:, 0:C], op=ALU.mult)
                # sim = dots * y + mask ; ex = exp(sim - 1)
                sim = sb.tile([P, Cmax], f32, tag="sim")
                nc.vector.tensor_tensor(out=sim[:, 0:C], in0=dots[:, 0:C],
                                        in1=y[:, 0:C], op=ALU.mult)
                nc.vector.tensor_tensor(out=sim[:, 0:C], in0=sim[:, 0:C],
                                        in1=mneg_all[:, base:base + C],
                                        op=ALU.add)
                ex = sb.tile([P, Cmax], f32, tag="ex")
                nc.scalar.activation(out=ex[:, 0:C], in_=sim[:, 0:C],
                                     func=AF.Exp, bias=neg1[:], scale=1.0)
                # per-group segment sum, reciprocal, weight scale
                S = sb.tile([P, MAXG], f32, tag="S")
                for n, g in enumerate(groups):
                    go = woffs[g] - base
                    w = Wd[g]
                    nc.vector.tensor_reduce(
                        out=S[:, n:n + 1],
                        in_=ex[:, go:go + w].rearrange("p (o w) -> p o w", o=1),
                        axis=mybir.AxisListType.X, op=ALU.add)
                nc.vector.tensor_scalar_add(out=S[:, 0:len(groups)],
                                            in0=S[:, 0:len(groups)],
                                            scalar1=float(TINY))
                r = sb.tile([P, MAXG], f32, tag="r")
                nc.vector.reciprocal_approx_fast(out=r[:, 0:len(groups)],
                                                 in_=S[:, 0:len(groups)])
                wq = wqp.tile([P, Cmax], f32, tag="wq")
                for n, g in enumerate(groups):
                    go = woffs[g] - base
                    w = Wd[g]
                    nc.vector.tensor_scalar_mul(
                        out=wq[:, go:go + w], in0=ex[:, go:go + w],
                        scalar1=r[:, n:n + 1])
                stB[ci] = (groups, x2q, wq)

            def stage_mac(ci):
                groups, x2q, wq = stB.pop(ci)
                g0 = groups[0]
                base = woffs[g0]
                ng = len(groups)
                agg = ps.tile([P, MAXG * D], f32, space="PSUM")
                nmm = sum(max(Ksched[g], 1) for g in groups)
                mi = 0
                for n, g in enumerate(groups):
                    go = woffs[g] - base
                    K = Ksched[g]
                    if K == 0:
                        dk0 = dg.tile([P, P], bf16, tag="dk0")
                        nc.vector.memset(dk0[:], 0.0)
                        nc.tensor.matmul(agg[:, n * D:(n + 1) * D], lhsT=dk0[:],
                                         rhs=x2q[:, go * D:(go + 1) * D],
                                         start=(mi == 0), stop=(mi == nmm - 1))
                        mi += 1
                        continue
                    dk = dg.tile([P, Kmax * P], bf16, tag="dk")
                    nc.gpsimd.tensor_tensor(
                        out=dk[:, 0:K * P].rearrange("p (k q) -> p k q", q=P),
                        in0=idm[:].rearrange("p (o q) -> p o q", o=1
                                             ).broadcast_to((P, K, P)),
                        in1=wq[:, go + 1:go + 1 + K].rearrange(
                            "p (k o) -> p k o", o=1).broadcast_to((P, K, P)),
                        op=ALU.mult)
                    for k in range(1, K + 1):
                        nc.tensor.matmul(agg[:, n * D:(n + 1) * D],
                                         lhsT=dk[:, (k - 1) * P:k * P],
                                         rhs=x2q[:, (go + k) * D:(go + k + 1) * D],
                                         start=(mi == 0), stop=(mi == nmm - 1))
                        mi += 1
                stC[ci] = (groups, agg)

            def stage_out(ci):
                groups, agg = stC.pop(ci)
                g0 = groups[0]
                ng = len(groups)
                out_sb = sb.tile([P, MAXG * D], f32, tag="outt")
                nc.vector.tensor_tensor(
                    out=out_sb[:, 0:ng * D], in0=agg[:, 0:ng * D],
                    in1=gates[:, g0 * P:(g0 + ng) * P], op=ALU.mult)
                nc.sync.dma_start(
                    out=out[g0 * P:(g0 + ng) * P, :].rearrange(
                        "(n p) d -> p n d", p=P),
                    in_=out_sb[:, 0:ng * D].rearrange("p (n d) -> p n d", d=D))

            for ci, groups in enumerate(chunks):
                g0 = groups[0]
                ng = len(groups)
                # gate unit: sigmoid(x) = 0.5*tanh(x/2) + 0.5, batched
                gps = psg.tile([P, MAXG * P], f32, space="PSUM")
                for n, g in enumerate(groups):
                    nc.tensor.matmul(gps[:, n * P:(n + 1) * P],
                                     lhsT=xnt_all[:, g * P:(g + 1) * P],
                                     rhs=wgt_sb[:], start=True, stop=True)
                ge = sb.tile([P, MAXG * P], bf16, tag="ge")
                nc.scalar.activation(out=ge[:, 0:ng * P], in_=gps[:, 0:ng * P],
                                     func=AF.Tanh, bias=0.0, scale=0.5)
                nc.vector.tensor_scalar(out=gates[:, g0 * P:(g0 + ng) * P],
                                        in0=ge[:, 0:ng * P], scalar1=0.5,
                                        scalar2=0.5, op0=ALU.mult, op1=ALU.add)

                base = woffs[g0]
                C = woffs[groups[-1] + 1] - base
                x2q = x2p.tile([P, Cmax * D], bf16, tag="x2")
                nc.sync.dma_start(out=x2q[:, 0:C * D],
                                  in_=x2s[:, base * D:(base + C) * D])
                stA[ci] = (groups, x2q)
                if ci - 1 in stA:
                    stage_front(ci - 1)
                if ci - 2 in stB:
                    stage_mac(ci - 2)
                if ci - 3 in stC:
                    stage_out(ci - 3)
            n = len(chunks)
            for j in range(max(0, n - 3), n):
                if j in stA:
                    stage_front(j)
            for j in range(max(0, n - 3), n):
                if j in stB:
                    stage_mac(j)
            for j in range(max(0, n - 3), n):
                if j in stC:
                    stage_out(j)
    nc.compile()
    return nc


def kernel(X_h_1, X_h_2, X_n_1, cross_indices, W_gate):
    global LAST_EXEC_NS
    from concourse.bass_utils import run_bass_kernel_spmd

    per_core, meta = _prep(X_h_1, X_h_2, X_n_1, cross_indices, W_gate)
    nc = _build(meta["Ksched"], meta["sumW"])

    in_maps = []
    for c in range(NCORES):
        pc = per_core[c]
        in_maps.append(dict(x2s=pc["x2s"], mnegs=pc["mneg_all"],
                            xnt=pc["xnt"], wgt=meta["wgt"], idmd=meta["idm"]))

    trace = bool(int(os.environ.get("BASS_KERNEL_TRACE", "0")))
    try:
        res = run_bass_kernel_spmd(nc, in_maps, list(range(NCORES)),
                                   trace=trace)
    except ModuleNotFoundError:
        res = run_bass_kernel_spmd(nc, in_maps, list(range(NCORES)),
                                   trace=False)
    LAST_EXEC_NS = res.exec_time_ns
    globals()["LAST_RES"] = res

    node_order_p = meta["node_order_p"]
    deg = meta["deg"]
    out_full = np.zeros((N1, D), dtype=np.float32)
    for c in range(NCORES):
        rows = res.results[c]["out"]
        for i in range(GPC):
            g = i * NCORES + c
            nodes = node_order_p[g * P:(g + 1) * P]
            vn = nodes >= 0
            out_full[nodes[vn]] = rows[i * P:(i + 1) * P][vn]
    out_full[deg == 0] = 0.0
    return out_full
